# revision 1
# baseline (speedup 1.0000x reference)
"""DirectionalGAT message-passing kernel for 8 Trainium2 NeuronCores.

Self-contained: host-side index marshaling + Bass/Tile program + PJRT SPMD run.

Strategy
--------
Node range [c*6250, (c+1)*6250) is owned by core c. Edges are replicated into
two sorted copies: D-copy grouped by dest block (computes fwd attention +
aggregation), S-copy grouped by src block (rev gate + aggregation), so every
segment reduction is core-local (no collectives in the edge phase). Per-node
first-layer projections are precomputed (node phase) and AllGathered so the
per-edge MLP becomes gather + PSUM adds; segment softmax/sums are masked
matmuls over 128-node blocks. The GRU update phase is node-parallel.
"""
import math
import os
import sys
import time

for _p in ("/opt/trn_rl_repo", os.path.expanduser("~/.axon_site/_ro/trn_rl_repo")):
    if os.path.isdir(_p) and _p not in sys.path:
        sys.path.insert(0, _p)

import numpy as np

NC, P = 8, 128

# ----------------------------------------------------------------- host prep


class _Cfg:
    def __init__(self, N, E, H=96, S=48, F=24):
        assert N % NC == 0
        self.N, self.E, self.H, self.S, self.F = N, E, H, S, F
        self.NPC = N // NC
        self.NBLK = (self.NPC + P - 1) // P
        self.NSLOT = self.NBLK * P

    def table_row(self, n):
        owner = n // self.NPC
        return owner * self.NSLOT + (n - owner * self.NPC)


def _build_copy(cfg, own, rand, ef, tb=None):
    NBLK, NPC = cfg.NBLK, cfg.NPC
    owner = own // NPC
    blk = (own - owner * NPC) // P
    slot = (own - owner * NPC) % P
    key = owner * NBLK + blk
    order = np.argsort(key, kind="stable")
    counts = np.bincount(key, minlength=NC * NBLK)
    if tb is None:
        tb = int((counts.max() + P - 1) // P)
    T = NBLK * tb
    gidx = np.zeros((NC, P, T), np.int32)
    vidx = np.zeros((NC, P, T), np.int32)
    dloc = np.full((NC, P, T), 999.0, np.float32)
    efa = np.zeros((NC, cfg.F + 1, T * P), np.float32)
    efa[:, cfg.F, :] = 1.0
    orig = np.full((NC, P, T), -1, np.int64)
    gid_all = cfg.table_row(rand)
    starts = np.zeros(NC * NBLK + 1, np.int64)
    np.cumsum(counts, out=starts[1:])
    for c in range(NC):
        for b in range(NBLK):
            k = c * NBLK + b
            eids = order[starts[k]:starts[k + 1]]
            cnt = len(eids)
            assert cnt <= tb * P, f"block overflow {cnt} > {tb*P}"
            tt = np.arange(cnt) // P + b * tb
            pp = np.arange(cnt) % P
            gidx[c, pp, tt] = gid_all[eids]
            vidx[c, pp, tt] = b * P + slot[eids]
            dloc[c, pp, tt] = slot[eids].astype(np.float32)
            efa[c, :cfg.F, tt * P + pp] = ef[eids]
            orig[c, pp, tt] = eids
    return dict(gidx=gidx, vidx=vidx, dloc=dloc, efa=efa, orig=orig, TB=tb, T=T)


def _signsplit(W1, b1, W2):
    w2 = W2[0]
    sgn = np.sign(w2)
    sgn[sgn == 0] = 1.0
    perm = np.argsort((sgn < 0).astype(np.int32), kind="stable")
    W1s = (W1 * np.abs(w2)[:, None])[perm]
    b1s = (b1 * np.abs(w2))[perm]
    return W1s.astype(np.float32), b1s.astype(np.float32), int((sgn > 0).sum())


def _prep_all(cfg, inp):
    H, S, F = cfg.H, cfg.S, cfg.F
    x = np.asarray(inp["x"], np.float32)
    x_s = np.asarray(inp["x_s"], np.float32)
    ef = np.asarray(inp["edge_features"], np.float32)
    ei = np.asarray(inp["edge_index"])
    src, dst = ei[0].astype(np.int64), ei[1].astype(np.int64)

    dcopy = _build_copy(cfg, dst, src, ef)
    scopy = _build_copy(cfg, src, dst, ef)
    tb = max(dcopy["TB"], scopy["TB"])
    if dcopy["TB"] != tb:
        dcopy = _build_copy(cfg, dst, src, ef, tb=tb)
    if scopy["TB"] != tb:
        scopy = _build_copy(cfg, src, dst, ef, tb=tb)

    fW1, fb1, fk = _signsplit(np.asarray(inp["fwd_W1"], np.float32),
                              np.asarray(inp["fwd_b1"], np.float32),
                              np.asarray(inp["fwd_W2"], np.float32))
    rW1, rb1, rk = _signsplit(np.asarray(inp["rev_W1"], np.float32),
                              np.asarray(inp["rev_b1"], np.float32),
                              np.asarray(inp["rev_W2"], np.float32))
    fb2 = float(np.asarray(inp["fwd_b2"]).ravel()[0])
    rb2 = float(np.asarray(inp["rev_b2"]).ravel()[0])

    def blocks(W, b):
        return dict(
            uT=np.ascontiguousarray(W[:, 0:H].T),
            usT=np.ascontiguousarray(W[:, 2 * H:2 * H + S].T),
            vT=np.ascontiguousarray(W[:, H:2 * H].T),
            vsT=np.ascontiguousarray(W[:, 2 * H + S:2 * H + 2 * S].T),
            eT=np.concatenate([W[:, 2 * H + 2 * S:].T, b[None, :]], 0),
        )

    wd = blocks(fW1, fb1)
    wd.update(k=fk, b2=fb2)
    ws = blocks(rW1, rb1)
    ws.update(k=rk, b2=rb2)

    def gru_pack(W1, b1, W2, b2):
        w1T = np.zeros((H, 9 * H), np.float32)
        for i in range(3):
            for j in range(3):
                w1T[:, (i * 3 + j) * H:(i * 3 + j + 1) * H] = \
                    W1[H * i:H * (i + 1), H * j:H * (j + 1)].T
        b1c = np.stack([b1[H * i:H * (i + 1)] for i in range(3)], 1)
        w2T = np.zeros((H, 3 * H), np.float32)
        for i in range(3):
            w2T[:, i * H:(i + 1) * H] = W2[:, H * i:H * (i + 1)].T
        return w1T, b1c, w2T, np.asarray(b2, np.float32)[:, None]

    gru = {g: gru_pack(np.asarray(inp[f"{g}_W1"], np.float32),
                       np.asarray(inp[f"{g}_b1"], np.float32),
                       np.asarray(inp[f"{g}_W2"], np.float32),
                       np.asarray(inp[f"{g}_b2"], np.float32))
           for g in ("reset", "update", "cand")}

    xpad = np.zeros((NC, cfg.NSLOT, H), np.float32)
    xspad = np.zeros((NC, cfg.NSLOT, S), np.float32)
    for c in range(NC):
        xpad[c, :cfg.NPC] = x[c * cfg.NPC:(c + 1) * cfg.NPC]
        xspad[c, :cfg.NPC] = x_s[c * cfg.NPC:(c + 1) * cfg.NPC]

    meta = dict(TB=tb, T=cfg.NBLK * tb, src=src, dst=dst)
    return dcopy, scopy, wd, ws, gru, xpad, xspad, meta


def _postprocess(cfg, dcopy, scopy, meta, e_out, s_out, w_out, upd, zg, rg):
    dst = meta["dst"]
    NPC = cfg.NPC
    fwd_w = np.zeros(cfg.E, np.float32)
    rev_w = np.zeros(cfg.E, np.float32)
    for c in range(NC):
        o = dcopy["orig"][c]
        m = o >= 0
        eids = o[m]
        dd = dst[eids]
        s_vals = s_out[c][(dd - c * NPC) % P, (dd - c * NPC) // P]
        fwd_w[eids] = e_out[c][m] / (s_vals + 1e-9)
        o2 = scopy["orig"][c]
        m2 = o2 >= 0
        rev_w[o2[m2]] = w_out[c][m2]
    update = np.concatenate([u[:NPC] for u in upd], 0)
    z_gate = np.concatenate([z[:NPC] for z in zg], 0)
    r_gate = np.concatenate([r[:NPC] for r in rg], 0)
    return update, fwd_w, rev_w, z_gate, r_gate


# ------------------------------------------------------- walrus wait splitter


def _fix_multiwait(nc, mybir):
    """This container's walrus rejects >1 sync wait per instruction; hoist
    extra waits onto single-wait EventSemaphore nops inserted just before."""
    ctr = 0
    for f in nc.m.functions:
        for bb in f.blocks:
            out, changed = [], False
            for inst in list(bb.instructions):
                si = inst.sync_info
                waits = list(si.on_wait) if si is not None else []
                if len(waits) > 1:
                    changed = True
                    for w in waits[:-1]:
                        ctr += 1
                        nop = mybir.InstEventSemaphore(
                            name=f"WSPLIT-{ctr}", ins=[], outs=[])
                        nop.engine = inst.engine
                        nop.sync_info = mybir.SyncInfo(on_wait=[w], on_update=[])
                        out.append(nop)
                    inst.sync_info = mybir.SyncInfo(
                        on_wait=[waits[-1]], on_update=list(si.on_update))
                out.append(inst)
            if changed:
                bb.instructions = out


# ----------------------------------------------------------- device program


def _build_nc(cfg, TB, fk, rk, fb2, rb2, repeat=1):
    import concourse.bass as bass
    import concourse.mybir as mybir
    from concourse.tile import TileContext
    from concourse.masks import make_identity

    F32, I32 = mybir.dt.float32, mybir.dt.int32
    AF = mybir.ActivationFunctionType
    OP = mybir.AluOpType

    H, S, F = cfg.H, cfg.S, cfg.F
    HH = 2 * H
    GW = HH + H
    EFR = F + 1
    NBLK, NSLOT = cfg.NBLK, cfg.NSLOT
    T = NBLK * TB
    inv_sqrt_h = 1.0 / math.sqrt(H)

    nc = bass.Bass("TRN2", target_bir_lowering=False, debug=False, num_devices=NC)

    def din(name, shape, dt=F32):
        return nc.dram_tensor(name, shape, dt, kind="ExternalInput")

    def dout(name, shape, dt=F32):
        return nc.dram_tensor(name, shape, dt, kind="ExternalOutput")

    x_own = din("x_own", [NSLOT, H])
    xs_own = din("xs_own", [NSLOT, S])
    io = {}
    for k in ("d", "s"):
        io[f"gidx_{k}"] = din(f"gidx_{k}", [P, T], I32)
        io[f"vidx_{k}"] = din(f"vidx_{k}", [P, T], I32)
        io[f"dloc_{k}"] = din(f"dloc_{k}", [P, T])
        io[f"ef_{k}"] = din(f"ef_{k}", [EFR, T * P])
        for w, shp in [("uT", [H, HH]), ("usT", [S, HH]), ("vT", [H, HH]),
                       ("vsT", [S, HH]), ("eT", [EFR, HH])]:
            io[f"w{k}_{w}"] = din(f"w{k}_{w}", shp)
    gru_w1T = din("gru_w1T", [H, 27 * H])
    gru_b1 = din("gru_b1", [H, 9])
    gru_w2T = din("gru_w2T", [H, 9 * H])
    gru_b2 = din("gru_b2", [H, 3])

    e_out = dout("e_out", [P, T])
    s_out = dout("s_out", [P, NBLK])
    w_out = dout("w_out", [P, T])
    upd_o = dout("upd_o", [NSLOT, H])
    zg_o = dout("zg_o", [NSLOT, H])
    rg_o = dout("rg_o", [NSLOT, H])

    gt_loc = {k: nc.dram_tensor(f"gt_loc_{k}", [NSLOT, GW], F32) for k in ("d", "s")}
    gt_full = {k: nc.dram_tensor(f"gt_full_{k}", [NC * NSLOT, GW], F32,
                                 addr_space="Shared") for k in ("d", "s")}
    ot = {k: nc.dram_tensor(f"ot_{k}", [NSLOT, HH], F32) for k in ("d", "s")}
    msT = {k: nc.dram_tensor(f"msT_{k}", [H, NSLOT], F32) for k in ("d", "s")}

    with TileContext(nc) as tc:
        with tc.tile_pool(name="const", bufs=1) as cpool:
            ident = cpool.tile([P, P], F32)
            make_identity(nc, ident[:])
            iota_i = cpool.tile([P, P], I32)
            nc.gpsimd.iota(iota_i[:], pattern=[[1, P]], base=0, channel_multiplier=0)
            iota_f = cpool.tile([P, P], F32)
            nc.vector.tensor_copy(iota_f[:], iota_i[:])
            wsb = {}
            for k in ("d", "s"):
                for w, shp in [("uT", [H, HH]), ("usT", [S, HH]), ("vT", [H, HH]),
                               ("vsT", [S, HH]), ("eT", [EFR, HH])]:
                    t_ = cpool.tile(shp, F32, tag=f"w{k}{w}")
                    nc.sync.dma_start(out=t_[:], in_=io[f"w{k}_{w}"][:])
                    wsb[f"{k}_{w}"] = t_
            g_w1T = cpool.tile([H, 27 * H], F32)
            nc.sync.dma_start(out=g_w1T[:], in_=gru_w1T[:])
            g_b1 = cpool.tile([H, 9], F32)
            nc.sync.dma_start(out=g_b1[:], in_=gru_b1[:])
            g_w2T = cpool.tile([H, 9 * H], F32)
            nc.sync.dma_start(out=g_w2T[:], in_=gru_w2T[:])
            g_b2 = cpool.tile([H, 3], F32)
            nc.sync.dma_start(out=g_b2[:], in_=gru_b2[:])

            # ---------------- node phase: per-node projections ----------------
            with tc.tile_pool(name="np_sb", bufs=3) as npool, \
                 tc.tile_pool(name="np_ps", bufs=2, space="PSUM") as npp, \
                 tc.tile_pool(name="np_ps2", bufs=2, space="PSUM") as npp2:
                for ch in range(NBLK):
                    r0 = ch * P
                    xc = npool.tile([P, H], F32, tag="xc")
                    nc.sync.dma_start(out=xc[:], in_=x_own[r0:r0 + P, :])
                    xsc = npool.tile([P, S], F32, tag="xsc")
                    nc.sync.dma_start(out=xsc[:], in_=xs_own[r0:r0 + P, :])
                    xT_ps = npp.tile([H, P], F32, tag="tr")
                    nc.tensor.transpose(out=xT_ps[:], in_=xc[:], identity=ident[:])
                    xT = npool.tile([H, P], F32, tag="xT")
                    nc.vector.tensor_copy(xT[:], xT_ps[:])
                    xsT_ps = npp.tile([S, P], F32, tag="tr2")
                    nc.tensor.transpose(out=xsT_ps[:], in_=xsc[:], identity=ident[:])
                    xsT = npool.tile([S, P], F32, tag="xsT")
                    nc.vector.tensor_copy(xsT[:], xsT_ps[:])
                    for k in ("d", "s"):
                        for kind in ("u", "v"):
                            pj = npp2.tile([P, HH], F32, tag="pj")
                            nc.tensor.matmul(pj[:], lhsT=xT[:],
                                             rhs=wsb[f"{k}_{kind}T"][:],
                                             start=True, stop=False)
                            nc.tensor.matmul(pj[:], lhsT=xsT[:],
                                             rhs=wsb[f"{k}_{kind}sT"][:],
                                             start=False, stop=True)
                            ps = npool.tile([P, HH], F32, tag="pjo")
                            nc.scalar.copy(ps[:], pj[:])
                            if kind == "u":
                                nc.sync.dma_start(out=gt_loc[k][r0:r0 + P, 0:HH], in_=ps[:])
                                nc.sync.dma_start(out=gt_loc[k][r0:r0 + P, HH:GW], in_=xc[:])
                            else:
                                nc.sync.dma_start(out=ot[k][r0:r0 + P, :], in_=ps[:])

            rg_all = [list(range(NC))]
            nc.gpsimd.collective_compute(
                "AllGather", OP.bypass, ins=[gt_loc["d"][:]], outs=[gt_full["d"][:]],
                replica_groups=rg_all)
            nc.gpsimd.collective_compute(
                "AllGather", OP.bypass, ins=[gt_loc["s"][:]], outs=[gt_full["s"][:]],
                replica_groups=rg_all)

            # ---------------- edge phases ----------------
            def edge_phase(k, softmax, kpos, b2val):
                with tc.tile_pool(name=f"e{k}_idx", bufs=1) as ipool, \
                     tc.tile_pool(name=f"e{k}_g", bufs=TB + 3) as gpool, \
                     tc.tile_pool(name=f"e{k}_v", bufs=3) as vpool, \
                     tc.tile_pool(name=f"e{k}_sm", bufs=3) as spool, \
                     tc.tile_pool(name=f"e{k}_ef", bufs=2) as efpool, \
                     tc.tile_pool(name=f"e{k}_ps", bufs=3, space="PSUM") as hpp, \
                     tc.tile_pool(name=f"e{k}_ag", bufs=2, space="PSUM") as app:
                    gidx_t = ipool.tile([P, T], I32)
                    nc.sync.dma_start(out=gidx_t[:], in_=io[f"gidx_{k}"][:])
                    vidx_t = ipool.tile([P, T], I32)
                    nc.sync.dma_start(out=vidx_t[:], in_=io[f"vidx_{k}"][:])
                    dloc_t = ipool.tile([P, T], F32)
                    nc.sync.dma_start(out=dloc_t[:], in_=io[f"dloc_{k}"][:])
                    sstage = ipool.tile([P, NBLK], F32)
                    b2t = ipool.tile([P, 1], F32)
                    nc.vector.memset(b2t[:], float(b2val))
                    eT_w = wsb[f"{k}_eT"]
                    for b in range(NBLK):
                        efc = efpool.tile([EFR, TB * P], F32, tag="ef")
                        nc.sync.dma_start(
                            out=efc[:], in_=io[f"ef_{k}"][:, b * TB * P:(b + 1) * TB * P])
                        sc_p = spool.tile([P, TB], F32, tag="scp")
                        sc_n = spool.tile([P, TB], F32, tag="scn")
                        g_tiles, st_tiles = [], []
                        for t in range(TB):
                            tau = b * TB + t
                            g = gpool.tile([P, GW], F32, tag="g")
                            nc.gpsimd.indirect_dma_start(
                                out=g[:], out_offset=None, in_=gt_full[k][:],
                                in_offset=bass.IndirectOffsetOnAxis(
                                    ap=gidx_t[:, tau:tau + 1], axis=0))
                            v = vpool.tile([P, HH], F32, tag="v")
                            nc.gpsimd.indirect_dma_start(
                                out=v[:], out_offset=None, in_=ot[k][:],
                                in_offset=bass.IndirectOffsetOnAxis(
                                    ap=vidx_t[:, tau:tau + 1], axis=0))
                            st = gpool.tile([P, P], F32, tag="st")
                            nc.vector.tensor_tensor(
                                out=st[:], in0=dloc_t[:, tau:tau + 1].to_broadcast([P, P]),
                                in1=iota_f[:], op=OP.is_equal)
                            h = hpp.tile([P, HH], F32, tag="h")
                            nc.tensor.matmul(h[:], lhsT=efc[:, t * P:(t + 1) * P],
                                             rhs=eT_w[:], start=True, stop=False)
                            nc.tensor.matmul(h[:], lhsT=ident[:], rhs=g[:, 0:HH],
                                             start=False, stop=False)
                            nc.tensor.matmul(h[:], lhsT=ident[:], rhs=v[:],
                                             start=False, stop=True)
                            scr = spool.tile([P, HH], F32, tag="scr")
                            nc.scalar.activation(
                                out=scr[:, 0:kpos], in_=h[:, 0:kpos], func=AF.Relu,
                                accum_out=sc_p[:, t:t + 1])
                            nc.scalar.activation(
                                out=scr[:, kpos:HH], in_=h[:, kpos:HH], func=AF.Relu,
                                accum_out=sc_n[:, t:t + 1])
                            g_tiles.append(g)
                            st_tiles.append(st)
                        raw = spool.tile([P, TB], F32, tag="raw")
                        nc.vector.tensor_tensor(out=raw[:], in0=sc_p[:], in1=sc_n[:],
                                                op=OP.subtract)
                        e_blk = spool.tile([P, TB], F32, tag="eblk")
                        if softmax:
                            rawb = spool.tile([P, TB], F32, tag="rawb")
                            nc.vector.tensor_scalar_add(rawb[:], raw[:], float(b2val))
                            pos = spool.tile([P, TB], F32, tag="lpos")
                            nc.vector.tensor_scalar_max(pos[:], rawb[:], 0.0)
                            mn = spool.tile([P, TB], F32, tag="lmin")
                            nc.vector.tensor_scalar(mn[:], rawb[:], 0.0, 0.01,
                                                    op0=OP.min, op1=OP.mult)
                            lr = spool.tile([P, TB], F32, tag="lr")
                            nc.vector.tensor_tensor(out=lr[:], in0=pos[:], in1=mn[:],
                                                    op=OP.add)
                            nc.scalar.activation(out=e_blk[:], in_=lr[:], func=AF.Exp,
                                                 scale=float(inv_sqrt_h))
                            nc.sync.dma_start(out=e_out[:, b * TB:(b + 1) * TB],
                                              in_=e_blk[:])
                        else:
                            nc.scalar.activation(out=e_blk[:], in_=raw[:],
                                                 func=AF.Sigmoid, bias=b2t[:])
                            nc.sync.dma_start(out=w_out[:, b * TB:(b + 1) * TB],
                                              in_=e_blk[:])
                        width = H + 1 if softmax else H
                        agg = app.tile([P, width], F32, tag="agg")
                        for t in range(TB):
                            m = spool.tile([P, width], F32, tag="m")
                            nc.vector.tensor_scalar_mul(
                                m[:, 0:H], g_tiles[t][:, HH:GW], e_blk[:, t:t + 1])
                            if softmax:
                                nc.vector.tensor_copy(m[:, H:H + 1], e_blk[:, t:t + 1])
                            nc.tensor.matmul(agg[:], lhsT=st_tiles[t][:], rhs=m[:],
                                             start=(t == 0), stop=(t == TB - 1))
                        ag_sb = spool.tile([P, H], F32, tag="agsb")
                        if softmax:
                            s_sb = spool.tile([P, 1], F32, tag="ssb")
                            nc.vector.tensor_copy(s_sb[:], agg[:, H:H + 1])
                            nc.vector.tensor_copy(sstage[:, b:b + 1], s_sb[:])
                            spl = spool.tile([P, 1], F32, tag="spl")
                            nc.vector.tensor_scalar_add(spl[:], s_sb[:], 1e-9)
                            rs = spool.tile([P, 1], F32, tag="rs")
                            nc.vector.reciprocal(rs[:], spl[:])
                            nc.vector.tensor_scalar_mul(ag_sb[:], agg[:, 0:H], rs[:])
                        else:
                            nc.vector.tensor_copy(ag_sb[:], agg[:, 0:H])
                        tp = hpp.tile([H, P], F32, tag="tp")
                        nc.tensor.transpose(out=tp[:], in_=ag_sb[:], identity=ident[:])
                        tp_sb = spool.tile([H, P], F32, tag="tpsb")
                        nc.scalar.copy(tp_sb[:], tp[:])
                        nc.sync.dma_start(out=msT[k][:, b * P:(b + 1) * P], in_=tp_sb[:])
                    if softmax:
                        nc.sync.dma_start(out=s_out[:], in_=sstage[:])

            # ---------------- GRU phase ----------------
            def gru_phase():
                with tc.tile_pool(name="g_sb", bufs=3) as gp, \
                     tc.tile_pool(name="g_ps", bufs=2, space="PSUM") as gps, \
                     tc.tile_pool(name="g_ps2", bufs=2, space="PSUM") as gps2:
                    for ch in range(NBLK):
                        r0 = ch * P
                        xc = gp.tile([P, H], F32, tag="xc")
                        nc.sync.dma_start(out=xc[:], in_=x_own[r0:r0 + P, :])
                        xT_ps = gps2.tile([H, P], F32, tag="xtr")
                        nc.tensor.transpose(out=xT_ps[:], in_=xc[:], identity=ident[:])
                        xT = gp.tile([H, P], F32, tag="xT")
                        nc.vector.tensor_copy(xT[:], xT_ps[:])
                        fT = gp.tile([H, P], F32, tag="fT")
                        nc.sync.dma_start(out=fT[:], in_=msT["d"][:, r0:r0 + P])
                        rT = gp.tile([H, P], F32, tag="rT")
                        nc.sync.dma_start(out=rT[:], in_=msT["s"][:, r0:r0 + P])

                        def gate_mlp(gi, blocks, act_func):
                            h1s = []
                            for i in range(3):
                                ps = gps.tile([H, P], F32, tag="g1")
                                for j in range(3):
                                    col = (gi * 9 + i * 3 + j) * H
                                    nc.tensor.matmul(
                                        ps[:], lhsT=g_w1T[:, col:col + H],
                                        rhs=blocks[j][:],
                                        start=(j == 0), stop=(j == 2))
                                h1 = gp.tile([H, P], F32, tag=f"h1_{i}")
                                nc.scalar.activation(
                                    out=h1[:], in_=ps[:], func=AF.Relu,
                                    bias=g_b1[:, gi * 3 + i:gi * 3 + i + 1])
                                h1s.append(h1)
                            ps2 = gps.tile([H, P], F32, tag="g2")
                            for i in range(3):
                                col = (gi * 3 + i) * H
                                nc.tensor.matmul(ps2[:], lhsT=g_w2T[:, col:col + H],
                                                 rhs=h1s[i][:],
                                                 start=(i == 0), stop=(i == 2))
                            o = gp.tile([H, P], F32, tag=f"go_{gi}")
                            nc.scalar.activation(out=o[:], in_=ps2[:], func=act_func,
                                                 bias=g_b2[:, gi:gi + 1])
                            return o

                        r_g = gate_mlp(0, [xT, fT, rT], AF.Sigmoid)
                        z_g = gate_mlp(1, [xT, fT, rT], AF.Sigmoid)
                        cx = gp.tile([H, P], F32, tag="cx")
                        nc.vector.tensor_tensor(out=cx[:], in0=r_g[:], in1=xT[:],
                                                op=OP.mult)
                        cand = gate_mlp(2, [cx, fT, rT], AF.Tanh)
                        t1 = gp.tile([H, P], F32, tag="t1")
                        nc.vector.tensor_tensor(out=t1[:], in0=cand[:], in1=xT[:],
                                                op=OP.subtract)
                        t2 = gp.tile([H, P], F32, tag="t2")
                        nc.vector.tensor_tensor(out=t2[:], in0=z_g[:], in1=t1[:],
                                                op=OP.mult)
                        u = gp.tile([H, P], F32, tag="u")
                        nc.vector.tensor_tensor(out=u[:], in0=xT[:], in1=t2[:],
                                                op=OP.add)
                        for val, dst_ in ((u, upd_o), (z_g, zg_o), (r_g, rg_o)):
                            ops_ = gps2.tile([P, H], F32, tag="otr")
                            nc.tensor.transpose(out=ops_[:], in_=val[:],
                                                identity=ident[0:H, 0:H])
                            osb = gp.tile([P, H], F32, tag="osb")
                            nc.vector.tensor_copy(osb[:], ops_[:])
                            nc.sync.dma_start(out=dst_[r0:r0 + P, :], in_=osb[:])

            def body():
                edge_phase("d", True, fk, fb2)
                edge_phase("s", False, rk, rb2)
                gru_phase()

            if repeat == 1:
                body()
            else:
                with tc.For_i(0, repeat, 1):
                    body()

    _fix_multiwait(nc, mybir)
    return nc


def _make_in_maps(cfg, dcopy, scopy, wd, ws, gru, xpad, xspad):
    w1T = np.concatenate([gru[g][0] for g in ("reset", "update", "cand")], 1)
    b1 = np.concatenate([gru[g][1] for g in ("reset", "update", "cand")], 1)
    w2T = np.concatenate([gru[g][2] for g in ("reset", "update", "cand")], 1)
    b2 = np.concatenate([gru[g][3] for g in ("reset", "update", "cand")], 1)
    maps = []
    for c in range(NC):
        m = dict(x_own=xpad[c], xs_own=xspad[c],
                 gru_w1T=w1T, gru_b1=b1, gru_w2T=w2T, gru_b2=b2)
        for k, cp, w in (("d", dcopy, wd), ("s", scopy, ws)):
            m[f"gidx_{k}"] = cp["gidx"][c]
            m[f"vidx_{k}"] = cp["vidx"][c]
            m[f"dloc_{k}"] = cp["dloc"][c]
            m[f"ef_{k}"] = cp["efa"][c]
            for nm in ("uT", "usT", "vT", "vsT", "eT"):
                m[f"w{k}_{nm}"] = w[nm]
        maps.append(m)
    return maps


# ----------------------------------------------------------- PJRT SPMD runner

_RUNNER_CACHE = {}


def _build_runner(nc):
    import jax
    from jax.sharding import Mesh, PartitionSpec
    from jax.experimental.shard_map import shard_map
    import concourse.mybir as mybir
    from concourse.bass2jax import (_bass_exec_p, partition_id_tensor,
                                    install_neuronx_cc_hook)

    install_neuronx_cc_hook()
    partition_name = nc.partition_id_tensor.name if nc.partition_id_tensor else None
    in_names, out_names, out_avals, zero_shapes = [], [], [], []
    for alloc in nc.m.functions[0].allocations:
        if not isinstance(alloc, mybir.MemoryLocationSet):
            continue
        name = alloc.memorylocations[0].name
        if alloc.kind == "ExternalInput":
            if name != partition_name:
                in_names.append(name)
        elif alloc.kind == "ExternalOutput":
            out_names.append(name)
            shape = tuple(alloc.tensor_shape)
            dtype = mybir.dt.np(alloc.dtype)
            out_avals.append(jax.core.ShapedArray(shape, dtype))
            zero_shapes.append((shape, dtype))
    n_params = len(in_names)
    n_outs = len(out_avals)
    in_names_all = in_names + out_names + ([partition_name] if partition_name else [])

    def _body(*args):
        operands = list(args)
        if partition_name is not None:
            operands.append(partition_id_tensor())
        outs = _bass_exec_p.bind(
            *operands,
            out_avals=tuple(out_avals), in_names=tuple(in_names_all),
            out_names=tuple(out_names), lowering_input_output_aliases=(),
            sim_require_finite=True, sim_require_nnan=True, nc=nc,
        )
        return tuple(outs)

    devices = jax.devices()[:NC]
    mesh = Mesh(np.asarray(devices), ("core",))
    donate = tuple(range(n_params, n_params + n_outs))
    sharded = jax.jit(
        shard_map(_body, mesh=mesh,
                  in_specs=(PartitionSpec("core"),) * (n_params + n_outs),
                  out_specs=(PartitionSpec("core"),) * n_outs, check_rep=False),
        donate_argnums=donate, keep_unused=True)

    def run(in_maps):
        per_core = [[np.asarray(m[nm]) for nm in in_names] for m in in_maps]
        concat_in = [np.concatenate([per_core[c][i] for c in range(NC)], axis=0)
                     for i in range(n_params)]
        zeros = [np.zeros((NC * s[0], *s[1:]), dt) for s, dt in zero_shapes]
        out_arrs = sharded(*concat_in, *zeros)
        import jax as _jax
        _jax.block_until_ready(out_arrs)
        return [
            {name: np.asarray(out_arrs[i]).reshape(NC, *zero_shapes[i][0])[c]
             for i, name in enumerate(out_names)}
            for c in range(NC)
        ]

    return run


# ------------------------------------------------------------------- entry


def kernel(**inputs):
    ei = np.asarray(inputs["edge_index"])
    N = inputs["x"].shape[0]
    E = ei.shape[1]
    cfg = _Cfg(N, E)
    dcopy, scopy, wd, ws, gru, xpad, xspad, meta = _prep_all(cfg, inputs)
    TB = meta["TB"]

    key = (N, E, TB, wd["k"], ws["k"], round(wd["b2"], 9), round(ws["b2"], 9))
    if key not in _RUNNER_CACHE:
        nc = _build_nc(cfg, TB, wd["k"], ws["k"], wd["b2"], ws["b2"])
        _RUNNER_CACHE[key] = _build_runner(nc)
    run = _RUNNER_CACHE[key]

    in_maps = _make_in_maps(cfg, dcopy, scopy, wd, ws, gru, xpad, xspad)
    res = run(in_maps)
    got = _postprocess(
        cfg, dcopy, scopy, meta,
        [res[c]["e_out"] for c in range(NC)],
        [res[c]["s_out"] for c in range(NC)],
        [res[c]["w_out"] for c in range(NC)],
        [res[c]["upd_o"] for c in range(NC)],
        [res[c]["zg_o"] for c in range(NC)],
        [res[c]["rg_o"] for c in range(NC)],
    )
    return got


# revision 7
# speedup vs baseline: 3.2912x; 3.2912x over previous
"""DirectionalGAT message-passing kernel for 8 Trainium2 NeuronCores.

Self-contained: host-side index marshaling + Bass/Tile program + PJRT SPMD run.

Strategy
--------
Node range [c*6250, (c+1)*6250) is owned by core c. Edges are replicated into
two sorted copies: D-copy grouped by dest block (computes fwd attention +
aggregation), S-copy grouped by src block (rev gate + aggregation), so every
segment reduction is core-local (no collectives in the edge phase). Per-node
first-layer projections are precomputed (node phase) and AllGathered so the
per-edge MLP becomes gather + PSUM adds; segment softmax/sums are masked
matmuls over 128-node blocks. The GRU update phase is node-parallel.
"""
import math
import os
import sys
import time

for _p in ("/opt/trn_rl_repo", os.path.expanduser("~/.axon_site/_ro/trn_rl_repo")):
    if os.path.isdir(_p) and _p not in sys.path:
        sys.path.insert(0, _p)

import numpy as np

NC, P = 8, 128

# ----------------------------------------------------------------- host prep


class _Cfg:
    def __init__(self, N, E, H=96, S=48, F=24):
        assert N % NC == 0
        self.N, self.E, self.H, self.S, self.F = N, E, H, S, F
        self.NPC = N // NC
        self.NBLK = (self.NPC + P - 1) // P
        self.NSLOT = self.NBLK * P

    def table_row(self, n):
        owner = n // self.NPC
        return owner * self.NSLOT + (n - owner * self.NPC)


def _build_copy(cfg, own, rand, ef, tb=None):
    NBLK, NPC = cfg.NBLK, cfg.NPC
    owner = own // NPC
    blk = (own - owner * NPC) // P
    slot = (own - owner * NPC) % P
    key = owner * NBLK + blk
    order = np.argsort(key, kind="stable")
    counts = np.bincount(key, minlength=NC * NBLK)
    if tb is None:
        tb = int((counts.max() + P - 1) // P)
    T = NBLK * tb
    gidx = np.zeros((NC, P, T), np.int32)
    vidx = np.zeros((NC, P, T), np.int32)
    dloc = np.full((NC, P, T), 999.0, np.float32)
    efa = np.zeros((NC, cfg.F + 1, T * P), np.float32)
    efa[:, cfg.F, :] = 1.0
    orig = np.full((NC, P, T), -1, np.int64)
    gid_all = cfg.table_row(rand)
    starts = np.zeros(NC * NBLK + 1, np.int64)
    np.cumsum(counts, out=starts[1:])
    for c in range(NC):
        for b in range(NBLK):
            k = c * NBLK + b
            eids = order[starts[k]:starts[k + 1]]
            cnt = len(eids)
            assert cnt <= tb * P, f"block overflow {cnt} > {tb*P}"
            tt = np.arange(cnt) // P + b * tb
            pp = np.arange(cnt) % P
            gidx[c, pp, tt] = gid_all[eids]
            vidx[c, pp, tt] = b * P + slot[eids]
            dloc[c, pp, tt] = slot[eids].astype(np.float32)
            efa[c, :cfg.F, tt * P + pp] = ef[eids]
            orig[c, pp, tt] = eids
    return dict(gidx=gidx, vidx=vidx, dloc=dloc, efa=efa, orig=orig, TB=tb, T=T)


def _signsplit(W1, b1, W2):
    w2 = W2[0]
    sgn = np.sign(w2)
    sgn[sgn == 0] = 1.0
    perm = np.argsort((sgn < 0).astype(np.int32), kind="stable")
    W1s = (W1 * np.abs(w2)[:, None])[perm]
    b1s = (b1 * np.abs(w2))[perm]
    return W1s.astype(np.float32), b1s.astype(np.float32), int((sgn > 0).sum())


def _prep_all(cfg, inp):
    H, S, F = cfg.H, cfg.S, cfg.F
    x = np.asarray(inp["x"], np.float32)
    x_s = np.asarray(inp["x_s"], np.float32)
    ef = np.asarray(inp["edge_features"], np.float32)
    ei = np.asarray(inp["edge_index"])
    src, dst = ei[0].astype(np.int64), ei[1].astype(np.int64)

    dcopy = _build_copy(cfg, dst, src, ef)
    scopy = _build_copy(cfg, src, dst, ef)
    tb = max(dcopy["TB"], scopy["TB"])
    if dcopy["TB"] != tb:
        dcopy = _build_copy(cfg, dst, src, ef, tb=tb)
    if scopy["TB"] != tb:
        scopy = _build_copy(cfg, src, dst, ef, tb=tb)

    fW1, fb1, fk = _signsplit(np.asarray(inp["fwd_W1"], np.float32),
                              np.asarray(inp["fwd_b1"], np.float32),
                              np.asarray(inp["fwd_W2"], np.float32))
    rW1, rb1, rk = _signsplit(np.asarray(inp["rev_W1"], np.float32),
                              np.asarray(inp["rev_b1"], np.float32),
                              np.asarray(inp["rev_W2"], np.float32))
    fb2 = float(np.asarray(inp["fwd_b2"]).ravel()[0])
    rb2 = float(np.asarray(inp["rev_b2"]).ravel()[0])

    def blocks(W, b):
        return dict(
            uT=np.ascontiguousarray(W[:, 0:H].T),
            usT=np.ascontiguousarray(W[:, 2 * H:2 * H + S].T),
            vT=np.ascontiguousarray(W[:, H:2 * H].T),
            vsT=np.ascontiguousarray(W[:, 2 * H + S:2 * H + 2 * S].T),
            eT=np.concatenate([W[:, 2 * H + 2 * S:].T, b[None, :]], 0),
        )

    wd = blocks(fW1, fb1)
    wd.update(k=fk, b2=fb2)
    ws = blocks(rW1, rb1)
    ws.update(k=rk, b2=rb2)

    def gru_pack(W1, b1, W2, b2):
        w1T = np.zeros((H, 9 * H), np.float32)
        for i in range(3):
            for j in range(3):
                w1T[:, (i * 3 + j) * H:(i * 3 + j + 1) * H] = \
                    W1[H * i:H * (i + 1), H * j:H * (j + 1)].T
        b1c = np.stack([b1[H * i:H * (i + 1)] for i in range(3)], 1)
        w2T = np.zeros((H, 3 * H), np.float32)
        for i in range(3):
            w2T[:, i * H:(i + 1) * H] = W2[:, H * i:H * (i + 1)].T
        return w1T, b1c, w2T, np.asarray(b2, np.float32)[:, None]

    gru = {g: gru_pack(np.asarray(inp[f"{g}_W1"], np.float32),
                       np.asarray(inp[f"{g}_b1"], np.float32),
                       np.asarray(inp[f"{g}_W2"], np.float32),
                       np.asarray(inp[f"{g}_b2"], np.float32))
           for g in ("reset", "update", "cand")}

    xpad = np.zeros((NC, cfg.NSLOT, H), np.float32)
    xspad = np.zeros((NC, cfg.NSLOT, S), np.float32)
    for c in range(NC):
        xpad[c, :cfg.NPC] = x[c * cfg.NPC:(c + 1) * cfg.NPC]
        xspad[c, :cfg.NPC] = x_s[c * cfg.NPC:(c + 1) * cfg.NPC]

    meta = dict(TB=tb, T=cfg.NBLK * tb, src=src, dst=dst)
    return dcopy, scopy, wd, ws, gru, xpad, xspad, meta


def _postprocess(cfg, dcopy, scopy, meta, e_out, s_out, w_out, upd, zg, rg):
    dst = meta["dst"]
    NPC = cfg.NPC
    fwd_w = np.zeros(cfg.E, np.float32)
    rev_w = np.zeros(cfg.E, np.float32)
    for c in range(NC):
        o = dcopy["orig"][c]
        m = o >= 0
        eids = o[m]
        dd = dst[eids]
        s_vals = s_out[c][(dd - c * NPC) % P, (dd - c * NPC) // P]
        fwd_w[eids] = e_out[c][m] / (s_vals + 1e-9)
        o2 = scopy["orig"][c]
        m2 = o2 >= 0
        rev_w[o2[m2]] = w_out[c][m2]
    update = np.concatenate([u[:NPC] for u in upd], 0)
    z_gate = np.concatenate([z[:NPC] for z in zg], 0)
    r_gate = np.concatenate([r[:NPC] for r in rg], 0)
    return update, fwd_w, rev_w, z_gate, r_gate


# ------------------------------------------------------- walrus wait splitter


def _fix_multiwait(nc, mybir):
    """This container's walrus rejects >1 sync wait per instruction; hoist
    extra waits onto single-wait EventSemaphore nops inserted just before."""
    ctr = 0
    for f in nc.m.functions:
        for bb in f.blocks:
            out, changed = [], False
            for inst in list(bb.instructions):
                si = inst.sync_info
                waits = list(si.on_wait) if si is not None else []
                if len(waits) > 1:
                    changed = True
                    for w in waits[:-1]:
                        ctr += 1
                        nop = mybir.InstEventSemaphore(
                            name=f"WSPLIT-{ctr}", ins=[], outs=[])
                        nop.engine = inst.engine
                        nop.sync_info = mybir.SyncInfo(on_wait=[w], on_update=[])
                        out.append(nop)
                    inst.sync_info = mybir.SyncInfo(
                        on_wait=[waits[-1]], on_update=list(si.on_update))
                out.append(inst)
            if changed:
                bb.instructions = out


# ----------------------------------------------------------- device program


def _build_nc(cfg, TB, fk, rk, fb2, rb2, repeat=1, for_hw=True):
    import concourse.bass as bass
    import concourse.mybir as mybir
    from concourse.tile import TileContext
    from concourse.masks import make_identity

    F32, I32 = mybir.dt.float32, mybir.dt.int32
    AF = mybir.ActivationFunctionType
    OP = mybir.AluOpType

    H, S, F = cfg.H, cfg.S, cfg.F
    HH = 2 * H
    GW = HH + H
    EFR = F + 1
    NBLK, NSLOT = cfg.NBLK, cfg.NSLOT
    T = NBLK * TB
    inv_sqrt_h = 1.0 / math.sqrt(H)

    nc = bass.Bass("TRN2", target_bir_lowering=False, debug=False, num_devices=NC)

    def din(name, shape, dt=F32):
        return nc.dram_tensor(name, shape, dt, kind="ExternalInput")

    def dout(name, shape, dt=F32):
        return nc.dram_tensor(name, shape, dt, kind="ExternalOutput")

    x_own = din("x_own", [NSLOT, H])
    xs_own = din("xs_own", [NSLOT, S])
    io = {}
    for k in ("d", "s"):
        io[f"gidx_{k}"] = din(f"gidx_{k}", [P, T], I32)
        io[f"dloc_{k}"] = din(f"dloc_{k}", [P, T])
        io[f"ef_{k}"] = din(f"ef_{k}", [EFR, T * P])
        for w, shp in [("uT", [H, HH]), ("usT", [S, HH]), ("vT", [H, HH]),
                       ("vsT", [S, HH]), ("eT", [EFR, HH])]:
            io[f"w{k}_{w}"] = din(f"w{k}_{w}", shp)
    gru_w1T = din("gru_w1T", [H, 27 * H])
    gru_b1 = din("gru_b1", [H, 9])
    gru_w2T = din("gru_w2T", [H, 9 * H])
    gru_b2 = din("gru_b2", [H, 3])

    e_out = dout("e_out", [P, T])
    s_out = dout("s_out", [P, NBLK])
    w_out = dout("w_out", [P, T])
    upd_o = dout("upd_o", [NSLOT, H])
    zg_o = dout("zg_o", [NSLOT, H])
    rg_o = dout("rg_o", [NSLOT, H])

    gt_loc = {k: nc.dram_tensor(f"gt_loc_{k}", [NSLOT, GW], F32) for k in ("d", "s")}
    gt_full = {k: nc.dram_tensor(f"gt_full_{k}", [NC * NSLOT, GW], F32,
                                 addr_space="Shared") for k in ("d", "s")}
    ot = {k: nc.dram_tensor(f"ot_{k}", [NSLOT, HH], F32) for k in ("d", "s")}
    msT = {k: nc.dram_tensor(f"msT_{k}", [H, NSLOT], F32) for k in ("d", "s")}

    with TileContext(nc) as tc:
        with tc.tile_pool(name="const", bufs=1) as cpool:
            ident = cpool.tile([P, P], F32)
            make_identity(nc, ident[:])
            iota_i = cpool.tile([P, P], I32)
            nc.gpsimd.iota(iota_i[:], pattern=[[1, P]], base=0, channel_multiplier=0)
            iota_f = cpool.tile([P, P], F32)
            nc.vector.tensor_copy(iota_f[:], iota_i[:])
            wsb = {}
            for k in ("d", "s"):
                for w, shp in [("uT", [H, HH]), ("usT", [S, HH]), ("vT", [H, HH]),
                               ("vsT", [S, HH]), ("eT", [EFR, HH])]:
                    t_ = cpool.tile(shp, F32, tag=f"w{k}{w}")
                    nc.sync.dma_start(out=t_[:], in_=io[f"w{k}_{w}"][:])
                    wsb[f"{k}_{w}"] = t_
            g_w1T = cpool.tile([H, 27 * H], F32)
            nc.sync.dma_start(out=g_w1T[:], in_=gru_w1T[:])
            g_b1 = cpool.tile([H, 9], F32)
            nc.sync.dma_start(out=g_b1[:], in_=gru_b1[:])
            g_w2T = cpool.tile([H, 9 * H], F32)
            nc.sync.dma_start(out=g_w2T[:], in_=gru_w2T[:])
            g_b2 = cpool.tile([H, 3], F32)
            nc.sync.dma_start(out=g_b2[:], in_=gru_b2[:])

            # ---------------- node phase: per-node projections ----------------
            with tc.tile_pool(name="np_sb", bufs=3) as npool, \
                 tc.tile_pool(name="np_ps", bufs=2, space="PSUM") as npp, \
                 tc.tile_pool(name="np_ps2", bufs=2, space="PSUM") as npp2:
                for ch in range(NBLK):
                    r0 = ch * P
                    xc = npool.tile([P, H], F32, tag="xc")
                    nc.sync.dma_start(out=xc[:], in_=x_own[r0:r0 + P, :])
                    xsc = npool.tile([P, S], F32, tag="xsc")
                    nc.sync.dma_start(out=xsc[:], in_=xs_own[r0:r0 + P, :])
                    xT_ps = npp.tile([H, P], F32, tag="tr")
                    nc.tensor.transpose(out=xT_ps[:], in_=xc[:], identity=ident[:])
                    xT = npool.tile([H, P], F32, tag="xT")
                    nc.vector.tensor_copy(xT[:], xT_ps[:])
                    xsT_ps = npp.tile([S, P], F32, tag="tr2")
                    nc.tensor.transpose(out=xsT_ps[:], in_=xsc[:], identity=ident[:])
                    xsT = npool.tile([S, P], F32, tag="xsT")
                    nc.vector.tensor_copy(xsT[:], xsT_ps[:])
                    for k in ("d", "s"):
                        for kind in ("u", "v"):
                            pj = npp2.tile([P, HH], F32, tag="pj")
                            nc.tensor.matmul(pj[:], lhsT=xT[:],
                                             rhs=wsb[f"{k}_{kind}T"][:],
                                             start=True, stop=False)
                            nc.tensor.matmul(pj[:], lhsT=xsT[:],
                                             rhs=wsb[f"{k}_{kind}sT"][:],
                                             start=False, stop=True)
                            ps = npool.tile([P, HH], F32, tag="pjo")
                            nc.scalar.copy(ps[:], pj[:])
                            if kind == "u":
                                nc.sync.dma_start(out=gt_loc[k][r0:r0 + P, 0:HH], in_=ps[:])
                                nc.sync.dma_start(out=gt_loc[k][r0:r0 + P, HH:GW], in_=xc[:])
                            else:
                                nc.sync.dma_start(out=ot[k][r0:r0 + P, :], in_=ps[:])

            rg_all = [list(range(NC))]
            nc.gpsimd.collective_compute(
                "AllGather", OP.bypass, ins=[gt_loc["d"][:]], outs=[gt_full["d"][:]],
                replica_groups=rg_all)
            nc.gpsimd.collective_compute(
                "AllGather", OP.bypass, ins=[gt_loc["s"][:]], outs=[gt_full["s"][:]],
                replica_groups=rg_all)

            # ---------------- edge phases ----------------
            def edge_phase(k, softmax, kpos, b2val):
                with tc.tile_pool(name=f"e{k}_idx", bufs=1) as ipool, \
                     tc.tile_pool(name=f"e{k}_g", bufs=TB + 3) as gpool, \
                     tc.tile_pool(name=f"e{k}_v", bufs=2) as vpool, \
                     tc.tile_pool(name=f"e{k}_sm", bufs=3) as spool, \
                     tc.tile_pool(name=f"e{k}_ef", bufs=2) as efpool, \
                     tc.tile_pool(name=f"e{k}_ps", bufs=2, space="PSUM") as hpp, \
                     tc.tile_pool(name=f"e{k}_ag", bufs=2, space="PSUM") as app:
                    gidx_t = ipool.tile([P, T], I32)
                    nc.sync.dma_start(out=gidx_t[:], in_=io[f"gidx_{k}"][:])
                    dloc_t = ipool.tile([P, T], F32)
                    nc.sync.dma_start(out=dloc_t[:], in_=io[f"dloc_{k}"][:])
                    sstage = ipool.tile([P, NBLK], F32)
                    b2t = ipool.tile([P, 1], F32)
                    nc.vector.memset(b2t[:], float(b2val))
                    eT_w = wsb[f"{k}_eT"]
                    for b in range(NBLK):
                        efc = efpool.tile([EFR, TB * P], F32, tag="ef")
                        nc.sync.dma_start(
                            out=efc[:], in_=io[f"ef_{k}"][:, b * TB * P:(b + 1) * TB * P])
                        otb = vpool.tile([P, HH], F32, tag="otb")
                        nc.sync.dma_start(out=otb[:], in_=ot[k][b * P:(b + 1) * P, :])
                        sc_p = spool.tile([P, TB], F32, tag="scp")
                        sc_n = spool.tile([P, TB], F32, tag="scn")
                        g_tiles, st_tiles = [], []
                        for t in range(TB):
                            tau = b * TB + t
                            g = gpool.tile([P, GW], F32, tag="g")
                            nc.gpsimd.indirect_dma_start(
                                out=g[:], out_offset=None, in_=gt_full[k][:],
                                in_offset=bass.IndirectOffsetOnAxis(
                                    ap=gidx_t[:, tau:tau + 1], axis=0))
                            st = gpool.tile([P, P], F32, tag="st")
                            nc.vector.tensor_tensor(
                                out=st[:], in0=dloc_t[:, tau:tau + 1].to_broadcast([P, P]),
                                in1=iota_f[:], op=OP.is_equal)
                            # owned-side term: S = st^T (PE transpose), then
                            # h += S^T-select of the block's V rows via matmul
                            s_ps = hpp.tile([P, P], F32, tag="str")
                            nc.tensor.transpose(out=s_ps[:], in_=st[:],
                                                identity=ident[:])
                            s_nb = spool.tile([P, P], F32, tag="snb")
                            if t % 2 == 0:
                                nc.vector.tensor_copy(s_nb[:], s_ps[:])
                            else:
                                nc.scalar.copy(s_nb[:], s_ps[:])
                            h = hpp.tile([P, HH], F32, tag="h")
                            nc.tensor.matmul(h[:], lhsT=efc[:, t * P:(t + 1) * P],
                                             rhs=eT_w[:], start=True, stop=False)
                            nc.tensor.matmul(h[:], lhsT=ident[:], rhs=g[:, 0:HH],
                                             start=False, stop=False)
                            nc.tensor.matmul(h[:], lhsT=s_nb[:], rhs=otb[:],
                                             start=False, stop=True)
                            scr = spool.tile([P, HH], F32, tag="scr")
                            nc.scalar.activation(
                                out=scr[:, 0:kpos], in_=h[:, 0:kpos], func=AF.Relu,
                                accum_out=sc_p[:, t:t + 1])
                            nc.scalar.activation(
                                out=scr[:, kpos:HH], in_=h[:, kpos:HH], func=AF.Relu,
                                accum_out=sc_n[:, t:t + 1])
                            g_tiles.append(g)
                            st_tiles.append(st)
                        raw = spool.tile([P, TB], F32, tag="raw")
                        nc.vector.tensor_tensor(out=raw[:], in0=sc_p[:], in1=sc_n[:],
                                                op=OP.subtract)
                        e_blk = spool.tile([P, TB], F32, tag="eblk")
                        if softmax:
                            rawb = spool.tile([P, TB], F32, tag="rawb")
                            nc.vector.tensor_scalar_add(rawb[:], raw[:], float(b2val))
                            pos = spool.tile([P, TB], F32, tag="lpos")
                            nc.vector.tensor_scalar_max(pos[:], rawb[:], 0.0)
                            mn = spool.tile([P, TB], F32, tag="lmin")
                            nc.vector.tensor_scalar(mn[:], rawb[:], 0.0, 0.01,
                                                    op0=OP.min, op1=OP.mult)
                            lr = spool.tile([P, TB], F32, tag="lr")
                            nc.vector.tensor_tensor(out=lr[:], in0=pos[:], in1=mn[:],
                                                    op=OP.add)
                            nc.scalar.activation(out=e_blk[:], in_=lr[:], func=AF.Exp,
                                                 scale=float(inv_sqrt_h))
                            nc.sync.dma_start(out=e_out[:, b * TB:(b + 1) * TB],
                                              in_=e_blk[:])
                        else:
                            nc.scalar.activation(out=e_blk[:], in_=raw[:],
                                                 func=AF.Sigmoid, bias=b2t[:])
                            nc.sync.dma_start(out=w_out[:, b * TB:(b + 1) * TB],
                                              in_=e_blk[:])
                        width = H + 1 if softmax else H
                        agg = app.tile([P, width], F32, tag="agg")
                        for t in range(TB):
                            m = spool.tile([P, width], F32, tag="m")
                            nc.vector.tensor_scalar_mul(
                                m[:, 0:H], g_tiles[t][:, HH:GW], e_blk[:, t:t + 1])
                            if softmax:
                                nc.vector.tensor_copy(m[:, H:H + 1], e_blk[:, t:t + 1])
                            nc.tensor.matmul(agg[:], lhsT=st_tiles[t][:], rhs=m[:],
                                             start=(t == 0), stop=(t == TB - 1))
                        ag_sb = spool.tile([P, H], F32, tag="agsb")
                        if softmax:
                            s_sb = spool.tile([P, 1], F32, tag="ssb")
                            nc.vector.tensor_copy(s_sb[:], agg[:, H:H + 1])
                            nc.vector.tensor_copy(sstage[:, b:b + 1], s_sb[:])
                            spl = spool.tile([P, 1], F32, tag="spl")
                            nc.vector.tensor_scalar_add(spl[:], s_sb[:], 1e-9)
                            rs = spool.tile([P, 1], F32, tag="rs")
                            nc.vector.reciprocal(rs[:], spl[:])
                            nc.vector.tensor_scalar_mul(ag_sb[:], agg[:, 0:H], rs[:])
                        else:
                            nc.vector.tensor_copy(ag_sb[:], agg[:, 0:H])
                        tp = hpp.tile([H, P], F32, tag="tp")
                        nc.tensor.transpose(out=tp[:], in_=ag_sb[:], identity=ident[:])
                        tp_sb = spool.tile([H, P], F32, tag="tpsb")
                        nc.scalar.copy(tp_sb[:], tp[:])
                        nc.sync.dma_start(out=msT[k][:, b * P:(b + 1) * P], in_=tp_sb[:])
                    if softmax:
                        nc.sync.dma_start(out=s_out[:], in_=sstage[:])

            # ---------------- GRU phase ----------------
            def gru_phase():
                with tc.tile_pool(name="g_sb", bufs=3) as gp, \
                     tc.tile_pool(name="g_ps", bufs=2, space="PSUM") as gps, \
                     tc.tile_pool(name="g_ps2", bufs=2, space="PSUM") as gps2:
                    for ch in range(NBLK):
                        r0 = ch * P
                        xc = gp.tile([P, H], F32, tag="xc")
                        nc.sync.dma_start(out=xc[:], in_=x_own[r0:r0 + P, :])
                        xT_ps = gps2.tile([H, P], F32, tag="xtr")
                        nc.tensor.transpose(out=xT_ps[:], in_=xc[:], identity=ident[:])
                        xT = gp.tile([H, P], F32, tag="xT")
                        nc.vector.tensor_copy(xT[:], xT_ps[:])
                        fT = gp.tile([H, P], F32, tag="fT")
                        nc.sync.dma_start(out=fT[:], in_=msT["d"][:, r0:r0 + P])
                        rT = gp.tile([H, P], F32, tag="rT")
                        nc.sync.dma_start(out=rT[:], in_=msT["s"][:, r0:r0 + P])

                        def gate_mlp(gi, blocks, act_func):
                            h1s = []
                            for i in range(3):
                                ps = gps.tile([H, P], F32, tag="g1")
                                for j in range(3):
                                    col = (gi * 9 + i * 3 + j) * H
                                    nc.tensor.matmul(
                                        ps[:], lhsT=g_w1T[:, col:col + H],
                                        rhs=blocks[j][:],
                                        start=(j == 0), stop=(j == 2))
                                h1 = gp.tile([H, P], F32, tag=f"h1_{i}")
                                nc.scalar.activation(
                                    out=h1[:], in_=ps[:], func=AF.Relu,
                                    bias=g_b1[:, gi * 3 + i:gi * 3 + i + 1])
                                h1s.append(h1)
                            ps2 = gps.tile([H, P], F32, tag="g2")
                            for i in range(3):
                                col = (gi * 3 + i) * H
                                nc.tensor.matmul(ps2[:], lhsT=g_w2T[:, col:col + H],
                                                 rhs=h1s[i][:],
                                                 start=(i == 0), stop=(i == 2))
                            o = gp.tile([H, P], F32, tag=f"go_{gi}")
                            nc.scalar.activation(out=o[:], in_=ps2[:], func=act_func,
                                                 bias=g_b2[:, gi:gi + 1])
                            return o

                        r_g = gate_mlp(0, [xT, fT, rT], AF.Sigmoid)
                        z_g = gate_mlp(1, [xT, fT, rT], AF.Sigmoid)
                        cx = gp.tile([H, P], F32, tag="cx")
                        nc.vector.tensor_tensor(out=cx[:], in0=r_g[:], in1=xT[:],
                                                op=OP.mult)
                        cand = gate_mlp(2, [cx, fT, rT], AF.Tanh)
                        t1 = gp.tile([H, P], F32, tag="t1")
                        nc.vector.tensor_tensor(out=t1[:], in0=cand[:], in1=xT[:],
                                                op=OP.subtract)
                        t2 = gp.tile([H, P], F32, tag="t2")
                        nc.vector.tensor_tensor(out=t2[:], in0=z_g[:], in1=t1[:],
                                                op=OP.mult)
                        u = gp.tile([H, P], F32, tag="u")
                        nc.vector.tensor_tensor(out=u[:], in0=xT[:], in1=t2[:],
                                                op=OP.add)
                        for val, dst_ in ((u, upd_o), (z_g, zg_o), (r_g, rg_o)):
                            ops_ = gps2.tile([P, H], F32, tag="otr")
                            nc.tensor.transpose(out=ops_[:], in_=val[:],
                                                identity=ident[0:H, 0:H])
                            osb = gp.tile([P, H], F32, tag="osb")
                            nc.vector.tensor_copy(osb[:], ops_[:])
                            nc.sync.dma_start(out=dst_[r0:r0 + P, :], in_=osb[:])

            def body():
                edge_phase("d", True, fk, fb2)
                edge_phase("s", False, rk, rb2)
                gru_phase()

            if repeat == 1:
                body()
            else:
                with tc.For_i(0, repeat, 1):
                    body()

    if for_hw:
        _fix_multiwait(nc, mybir)
    return nc


def _make_in_maps(cfg, dcopy, scopy, wd, ws, gru, xpad, xspad):
    w1T = np.concatenate([gru[g][0] for g in ("reset", "update", "cand")], 1)
    b1 = np.concatenate([gru[g][1] for g in ("reset", "update", "cand")], 1)
    w2T = np.concatenate([gru[g][2] for g in ("reset", "update", "cand")], 1)
    b2 = np.concatenate([gru[g][3] for g in ("reset", "update", "cand")], 1)
    maps = []
    for c in range(NC):
        m = dict(x_own=xpad[c], xs_own=xspad[c],
                 gru_w1T=w1T, gru_b1=b1, gru_w2T=w2T, gru_b2=b2)
        for k, cp, w in (("d", dcopy, wd), ("s", scopy, ws)):
            m[f"gidx_{k}"] = cp["gidx"][c]
            m[f"dloc_{k}"] = cp["dloc"][c]
            m[f"ef_{k}"] = cp["efa"][c]
            for nm in ("uT", "usT", "vT", "vsT", "eT"):
                m[f"w{k}_{nm}"] = w[nm]
        maps.append(m)
    return maps


# ----------------------------------------------------------- PJRT SPMD runner

_RUNNER_CACHE = {}


def _build_runner(nc):
    import jax
    from jax.sharding import Mesh, PartitionSpec
    from jax.experimental.shard_map import shard_map
    import concourse.mybir as mybir
    from concourse.bass2jax import (_bass_exec_p, partition_id_tensor,
                                    install_neuronx_cc_hook)

    install_neuronx_cc_hook()
    partition_name = nc.partition_id_tensor.name if nc.partition_id_tensor else None
    in_names, out_names, out_avals, zero_shapes = [], [], [], []
    for alloc in nc.m.functions[0].allocations:
        if not isinstance(alloc, mybir.MemoryLocationSet):
            continue
        name = alloc.memorylocations[0].name
        if alloc.kind == "ExternalInput":
            if name != partition_name:
                in_names.append(name)
        elif alloc.kind == "ExternalOutput":
            out_names.append(name)
            shape = tuple(alloc.tensor_shape)
            dtype = mybir.dt.np(alloc.dtype)
            out_avals.append(jax.core.ShapedArray(shape, dtype))
            zero_shapes.append((shape, dtype))
    n_params = len(in_names)
    n_outs = len(out_avals)
    in_names_all = in_names + out_names + ([partition_name] if partition_name else [])

    def _body(*args):
        operands = list(args)
        if partition_name is not None:
            operands.append(partition_id_tensor())
        outs = _bass_exec_p.bind(
            *operands,
            out_avals=tuple(out_avals), in_names=tuple(in_names_all),
            out_names=tuple(out_names), lowering_input_output_aliases=(),
            sim_require_finite=True, sim_require_nnan=True, nc=nc,
        )
        return tuple(outs)

    devices = jax.devices()[:NC]
    mesh = Mesh(np.asarray(devices), ("core",))
    donate = tuple(range(n_params, n_params + n_outs))
    sharded = jax.jit(
        shard_map(_body, mesh=mesh,
                  in_specs=(PartitionSpec("core"),) * (n_params + n_outs),
                  out_specs=(PartitionSpec("core"),) * n_outs, check_rep=False),
        donate_argnums=donate, keep_unused=True)

    def run(in_maps):
        per_core = [[np.asarray(m[nm]) for nm in in_names] for m in in_maps]
        concat_in = [np.concatenate([per_core[c][i] for c in range(NC)], axis=0)
                     for i in range(n_params)]
        zeros = [np.zeros((NC * s[0], *s[1:]), dt) for s, dt in zero_shapes]
        out_arrs = sharded(*concat_in, *zeros)
        import jax as _jax
        _jax.block_until_ready(out_arrs)
        return [
            {name: np.asarray(out_arrs[i]).reshape(NC, *zero_shapes[i][0])[c]
             for i, name in enumerate(out_names)}
            for c in range(NC)
        ]

    return run


# ------------------------------------------------------------------- entry


def kernel(**inputs):
    ei = np.asarray(inputs["edge_index"])
    N = inputs["x"].shape[0]
    E = ei.shape[1]
    cfg = _Cfg(N, E)
    dcopy, scopy, wd, ws, gru, xpad, xspad, meta = _prep_all(cfg, inputs)
    TB = meta["TB"]

    key = (N, E, TB, wd["k"], ws["k"], round(wd["b2"], 9), round(ws["b2"], 9))
    if key not in _RUNNER_CACHE:
        nc = _build_nc(cfg, TB, wd["k"], ws["k"], wd["b2"], ws["b2"])
        _RUNNER_CACHE[key] = _build_runner(nc)
    run = _RUNNER_CACHE[key]

    in_maps = _make_in_maps(cfg, dcopy, scopy, wd, ws, gru, xpad, xspad)
    res = run(in_maps)
    got = _postprocess(
        cfg, dcopy, scopy, meta,
        [res[c]["e_out"] for c in range(NC)],
        [res[c]["s_out"] for c in range(NC)],
        [res[c]["w_out"] for c in range(NC)],
        [res[c]["upd_o"] for c in range(NC)],
        [res[c]["zg_o"] for c in range(NC)],
        [res[c]["rg_o"] for c in range(NC)],
    )
    return got
